# revision 15
# baseline (speedup 1.0000x reference)
"""Trainium2 Bass kernel for nn_PriorW (Wishart-prior sampling).

Math per batch b (wdim=16):
  A  = tril(A_noise,-1) + diag(sqrt(chisq))        (lower-triangular)
  B  = inv(A),  scale_i = 1/||B[:,i]||_2
  M^T[j,i] = B[j,i]*scale_i
  w[b,n,:] = z[b,n,:] @ M^T                         (n = 4096 draws)

Device algorithm:
  A = D(I + N) with N = D^{-1} L strictly lower nilpotent (N^16 = 0), so
  (I+N)^{-1} = (I-N)(I-N^2)(I-N^4)(I-N^8) EXACTLY, i.e. P = (I+N)^{-1}
  comes from a short chain of 16x16 matrix products. Those run on the PE
  with 8 batches packed per 128x128 block-diagonal tile. Since
  B[:,i] = P[:,i]*r_i, the row normalization reduces to
  Mt[j,i] = P[j,i]/||P[:,i]|| — the final column norm is read off the
  diagonal of G = P^T P (one more PE matmul) and applied as a per-partition
  scale fused into the PSUM->SBUF copy of the main z-stream matmuls.

Sharding: pure data parallel over nbatch (1024 / 8 cores = 128 per core).
Group g in [0,16), slot b8 in [0,8): batch b = 16*b8 + g.
  zt row (g*128 + 16*b8 + j) = z[b, :, j]   (n contiguous -> fast DMA)
  wt row (g*128 + 16*b8 + i) = w[b, :, i]
"""
import os
import sys
import types

import numpy as np

for _p in ("/opt/trn_rl_repo", "/root/.axon_site/_ro/trn_rl_repo"):
    if _p not in sys.path:
        sys.path.append(_p)

import concourse.bass as bass  # noqa: E402
import concourse.tile as tile  # noqa: E402
from concourse import bacc, mybir  # noqa: E402
from concourse import bass_utils  # noqa: E402
from concourse.masks import make_identity  # noqa: E402

WD = 16          # wishart dim
BPC = 128        # batches per core
N = 4096         # draws per batch
NT = 512         # matmul moving-dim tile
NCORES = 8
NBATCH = BPC * NCORES
F32 = mybir.dt.float32
F32R = mybir.dt.float32r

USE_F32R = os.environ.get("PRIORW_F32R", "1") == "1"
ZDT = F32R if USE_F32R else F32

_PROGRAM_CACHE = {}


def _setup_trace_hooks():
    """Register the axon NTFF profile hook (missing antenv.axon_hooks shim)."""
    try:
        import antenv
        if "antenv.axon_hooks" not in sys.modules:
            hooks = types.ModuleType("antenv.axon_hooks")
            _h = [None]
            hooks.set_axon_ntff_profile_hook = lambda h: _h.__setitem__(0, h)
            hooks.get_axon_ntff_profile_hook = lambda: _h[0]
            sys.modules["antenv.axon_hooks"] = hooks
            antenv.axon_hooks = hooks
        from antenv.axon_hooks import set_axon_ntff_profile_hook
        from trn_agent_boot.trn_boot import _ntff_profile_via_ctypes
        hook = _ntff_profile_via_ctypes("/opt/axon/libaxon_pjrt.so")
        if hook is not None:
            set_axon_ntff_profile_hook(hook)
        bass_utils.upload_artifacts = lambda tmpdir: tmpdir  # no egress
        return True
    except Exception:
        return False


def _build_program():
    nc = bacc.Bacc("TRN2", target_bir_lowering=False)
    an_ext = nc.declare_dram_parameter("a_noise", [BPC, WD * WD], F32, isOutput=False)
    cs_ext = nc.declare_dram_parameter("chisq", [BPC, WD], F32, isOutput=False)
    zt_ext = nc.declare_dram_parameter("zt", [BPC * WD, N], ZDT, isOutput=False)
    wt_ext = nc.declare_dram_parameter("wt", [BPC * WD, N], F32, isOutput=True)
    # bounce buffer for the partition<->free shuffle of N: [b8, u, g, v]
    lt_dram = nc.dram_tensor("lt_scratch", [8, WD, WD, WD], F32)

    MULT = mybir.AluOpType.mult
    BYPASS = mybir.AluOpType.bypass

    with tile.TileContext(nc) as tc:
        with tc.tile_pool(name="pro", bufs=1) as pro, \
             tc.tile_pool(name="chain", bufs=2) as ch, \
             tc.tile_pool(name="zin", bufs=3) as zpool, \
             tc.tile_pool(name="wout", bufs=3) as wpool, \
             tc.tile_pool(name="cpsp", bufs=3, space="PSUM") as cpsp, \
             tc.tile_pool(name="mpsp", bufs=4, space="PSUM") as mpsp:

            # ---------- prologue: N = D^{-1} L (block-diag packed) ----------
            an = pro.tile([BPC, WD * WD], F32)
            nc.gpsimd.dma_start(out=an, in_=an_ext[:])
            cs = pro.tile([BPC, WD], F32)
            nc.gpsimd.dma_start(out=cs, in_=cs_ext[:])

            d = pro.tile([BPC, WD], F32)
            nc.scalar.sqrt(d, cs)
            r = pro.tile([BPC, WD], F32)
            nc.vector.reciprocal(r, d)

            # Ltil[b, 16u+v] = r_u * A[b,u,v], strictly-lower masked
            ltil = pro.tile([BPC, WD * WD], F32)
            for u in range(WD):
                eng = nc.vector if u % 2 == 0 else nc.gpsimd
                eng.tensor_scalar_mul(
                    ltil[:, WD * u:WD * (u + 1)],
                    an[:, WD * u:WD * (u + 1)],
                    r[:, u:u + 1])
            nc.gpsimd.affine_select(
                out=ltil, in_=ltil,
                compare_op=mybir.AluOpType.is_ge, fill=0.0,
                base=-1, pattern=[[1, WD], [-1, WD]], channel_multiplier=0)

            ident = pro.tile([BPC, BPC], F32)
            make_identity(nc, ident)

            # block-diag N tile for all 16 groups: [128, 16*128]
            nbd = pro.tile([BPC, 16 * BPC], F32)
            nc.vector.memset(nbd, 0.0)
            for b8 in range(8):
                psl = slice(WD * b8, WD * (b8 + 1))
                nc.gpsimd.dma_start(
                    out=lt_dram[b8].rearrange("u g v -> g u v"),
                    in_=ltil[psl, :].rearrange("g (u v) -> g u v", u=WD),
                )
            for b8 in range(8):
                psl = slice(WD * b8, WD * (b8 + 1))
                ndst = nbd[psl, :].rearrange("u (g v) -> u g v", g=16)
                nc.gpsimd.dma_start(
                    out=ndst[:, :, WD * b8:WD * (b8 + 1)],
                    in_=lt_dram[b8],
                )

            lhsT_all = pro.tile([BPC, 16 * BPC], ZDT)
            s_all = pro.tile([BPC, 16], F32)

            def mm(lhsT, rhs):
                ps = cpsp.tile([BPC, BPC], F32, tag="cps")
                nc.tensor.matmul(ps, lhsT=lhsT, rhs=rhs, start=True, stop=True)
                return ps

            def mm_sb(lhsT, rhs, out=None):
                ps = mm(lhsT, rhs)
                if out is None:
                    out = ch.tile([BPC, BPC], F32, tag="mmsb")
                nc.vector.tensor_copy(out, ps)
                return out

            def isub(x, tag):
                out = ch.tile([BPC, BPC], F32, tag=tag)
                nc.vector.tensor_sub(out, ident, x)
                return out

            def iadd(x, tag):
                out = ch.tile([BPC, BPC], F32, tag=tag)
                nc.vector.tensor_add(out, ident, x)
                return out

            for g in range(16):
                gsl = slice(BPC * g, BPC * (g + 1))
                ng = nbd[:, gsl]
                # N^T via PE transpose (block-diag transposes blockwise)
                tps = cpsp.tile([BPC, BPC], F32, tag="cps")
                nc.tensor.transpose(tps, ng, ident)
                tg = ch.tile([BPC, BPC], F32, tag="tg")
                nc.vector.tensor_copy(tg, tps)

                # ---- chain: P = (I-N)(I+N^2)(I+N^4)(I+N^8) = inv(I+N)
                # (alternating geometric series; exact since N^16 = 0)
                s1 = isub(ng, "s1")           # I - N
                s1t = isub(tg, "s1t")         # I - N^T
                n2 = mm_sb(tg, ng)            # N^2
                t2 = mm_sb(ng, tg)            # (N^2)^T
                a2 = iadd(n2, "a2")           # I + N^2
                a2t = iadd(t2, "a2t")
                n4 = mm_sb(t2, n2)            # N^4
                t4 = mm_sb(n2, t2)            # (N^4)^T
                a4 = iadd(n4, "a4")
                a4t = iadd(t4, "a4t")
                n8 = mm_sb(t4, n4)            # N^8
                a8 = iadd(n8, "a8")
                q2 = mm_sb(s1t, a2)           # (I-N)(I+N^2)
                q2t = mm_sb(a2, s1t)          # ^T
                q4 = mm_sb(q2t, a4)           # (I-N)(I+N^2)(I+N^4)
                q4t = mm_sb(a4, q2t)          # ^T
                p = mm_sb(q4t, a8, out=lhsT_all[:, gsl])   # P
                gram = mm(p, p)               # G = P^T P (PSUM)
                scr = ch.tile([BPC, BPC], F32, tag="scr")
                d2 = ch.tile([BPC, 1], F32, tag="d2")
                nc.vector.scalar_tensor_tensor(
                    out=scr, in0=gram, scalar=1.0, in1=ident,
                    op0=BYPASS, op1=MULT, accum_out=d2)
                ssq = ch.tile([BPC, 1], F32, tag="ssq")
                nc.scalar.sqrt(ssq, d2)
                nc.vector.reciprocal(s_all[:, g:g + 1], ssq)

                if g == 0 and os.environ.get("PRIORW_DEBUG"):
                    for nm, tl in [("dbg_tg", tg), ("dbg_n2", n2),
                                   ("dbg_t2", t2), ("dbg_n8", n8),
                                   ("dbg_q2", q2), ("dbg_s8", a8)]:
                        dt_ = nc.dram_tensor(nm, [BPC, BPC], F32)
                        nc.sync.dma_start(out=dt_[:], in_=tl)

                # ---- main stream for this group
                zin = zpool.tile([BPC, N], ZDT)
                nc.sync.dma_start(out=zin, in_=zt_ext[gsl, :])
                wout = wpool.tile([BPC, N], F32)
                lhs_g = lhsT_all[:, gsl]
                for t in range(N // NT):
                    csl = slice(NT * t, NT * (t + 1))
                    ps = mpsp.tile([BPC, NT], F32, tag="mps")
                    nc.tensor.matmul(ps, lhsT=lhs_g, rhs=zin[:, csl],
                                     start=True, stop=True)
                    nc.vector.tensor_scalar_mul(wout[:, csl], ps, s_all[:, g:g + 1])
                    if t == (N // NT) // 2 - 1:
                        nc.scalar.dma_start(
                            out=wt_ext[gsl, 0:N // 2], in_=wout[:, 0:N // 2])
                nc.scalar.dma_start(
                    out=wt_ext[gsl, N // 2:N], in_=wout[:, N // 2:N])

            if os.environ.get("PRIORW_DEBUG"):
                dbg_nbd = nc.dram_tensor("dbg_nbd", [BPC, 16 * BPC], F32)
                dbg_lhs = nc.dram_tensor("dbg_lhs", [BPC, 16 * BPC], F32)
                dbg_s = nc.dram_tensor("dbg_s", [BPC, 16], F32)
                nc.sync.dma_start(out=dbg_nbd[:], in_=nbd)
                nc.sync.dma_start(out=dbg_lhs[:], in_=lhsT_all)
                nc.sync.dma_start(out=dbg_s[:], in_=s_all)

    nc.compile()
    return nc


def _get_program():
    if "nc" not in _PROGRAM_CACHE:
        _PROGRAM_CACHE["nc"] = _build_program()
    return _PROGRAM_CACHE["nc"]


def kernel(A_noise, chisq, z):
    from concourse.bass_utils import run_bass_kernel_spmd

    A_noise = np.ascontiguousarray(A_noise, dtype=np.float32)
    chisq = np.ascontiguousarray(chisq, dtype=np.float32)
    z = np.ascontiguousarray(z, dtype=np.float32)

    trace = bool(os.environ.get("PRIORW_TRACE"))
    if trace:
        trace = _setup_trace_hooks()

    nc = _get_program()

    in_maps = []
    for c in range(NCORES):
        sl = slice(c * BPC, (c + 1) * BPC)
        # [128,4096,16] -> (b8, g, n, j) -> (g, b8, j, n) -> [2048, 4096]
        zt = np.ascontiguousarray(
            z[sl].reshape(8, 16, N, WD).transpose(1, 0, 3, 2)
        ).reshape(BPC * WD, N)
        in_maps.append({
            "a_noise": A_noise[sl].reshape(BPC, WD * WD),
            "chisq": chisq[sl],
            "zt": zt,
        })

    res = run_bass_kernel_spmd(nc, in_maps, list(range(NCORES)), trace=trace)
    if trace and res.exec_time_ns is not None:
        print(f"HW exec time: {res.exec_time_ns} ns")

    w = np.empty((NBATCH, N, WD), dtype=np.float32)
    for c in range(NCORES):
        wt = np.asarray(res.results[c]["wt"])
        w[c * BPC:(c + 1) * BPC] = (
            wt.reshape(16, 8, WD, N).transpose(1, 0, 3, 2).reshape(BPC, N, WD)
        )
    return w


# revision 18
# speedup vs baseline: 1.2808x; 1.2808x over previous
"""Trainium2 Bass kernel for nn_PriorW (Wishart-prior sampling).

Math per batch b (wdim=16):
  A  = tril(A_noise,-1) + diag(sqrt(chisq))        (lower-triangular)
  B  = inv(A),  scale_i = 1/||B[:,i]||_2
  M^T[j,i] = B[j,i]*scale_i
  w[b,n,:] = z[b,n,:] @ M^T                         (n = 4096 draws)

Device algorithm:
  A = D(I + N) with N = D^{-1} L strictly lower nilpotent (N^16 = 0), so
  (I+N)^{-1} = (I-N)(I-N^2)(I-N^4)(I-N^8) EXACTLY, i.e. P = (I+N)^{-1}
  comes from a short chain of 16x16 matrix products. Those run on the PE
  with 8 batches packed per 128x128 block-diagonal tile. Since
  B[:,i] = P[:,i]*r_i, the row normalization reduces to
  Mt[j,i] = P[j,i]/||P[:,i]|| — the final column norm is read off the
  diagonal of G = P^T P (one more PE matmul) and applied as a per-partition
  scale fused into the PSUM->SBUF copy of the main z-stream matmuls.

Sharding: pure data parallel over nbatch (1024 / 8 cores = 128 per core).
Group g in [0,16), slot b8 in [0,8): batch b = 16*b8 + g.
  zt row (g*128 + 16*b8 + j) = z[b, :, j]   (n contiguous -> fast DMA)
  wt row (g*128 + 16*b8 + i) = w[b, :, i]
"""
import os
import sys
import types

import numpy as np

for _p in ("/opt/trn_rl_repo", "/root/.axon_site/_ro/trn_rl_repo"):
    if _p not in sys.path:
        sys.path.append(_p)

import concourse.bass as bass  # noqa: E402
import concourse.tile as tile  # noqa: E402
from concourse import bacc, mybir  # noqa: E402
from concourse import bass_utils  # noqa: E402
from concourse.masks import make_identity  # noqa: E402

WD = 16          # wishart dim
BPC = 128        # batches per core
N = 4096         # draws per batch
NT = 512         # matmul moving-dim tile
NCORES = 8
NBATCH = BPC * NCORES
F32 = mybir.dt.float32
F32R = mybir.dt.float32r
F16 = mybir.dt.float16

USE_F32R = os.environ.get("PRIORW_F32R", "1") == "1"
ZDT = F32R if USE_F32R else F32

_PROGRAM_CACHE = {}


def _setup_trace_hooks():
    """Register the axon NTFF profile hook (missing antenv.axon_hooks shim)."""
    try:
        import antenv
        if "antenv.axon_hooks" not in sys.modules:
            hooks = types.ModuleType("antenv.axon_hooks")
            _h = [None]
            hooks.set_axon_ntff_profile_hook = lambda h: _h.__setitem__(0, h)
            hooks.get_axon_ntff_profile_hook = lambda: _h[0]
            sys.modules["antenv.axon_hooks"] = hooks
            antenv.axon_hooks = hooks
        from antenv.axon_hooks import set_axon_ntff_profile_hook
        from trn_agent_boot.trn_boot import _ntff_profile_via_ctypes
        hook = _ntff_profile_via_ctypes("/opt/axon/libaxon_pjrt.so")
        if hook is not None:
            set_axon_ntff_profile_hook(hook)
        bass_utils.upload_artifacts = lambda tmpdir: tmpdir  # no egress
        return True
    except Exception:
        return False


def _build_program():
    nc = bacc.Bacc("TRN2", target_bir_lowering=False)
    an_ext = nc.declare_dram_parameter("a_noise", [BPC, WD * WD], F32, isOutput=False)
    cs_ext = nc.declare_dram_parameter("chisq", [BPC, WD], F32, isOutput=False)
    zt_ext = nc.declare_dram_parameter("zt", [BPC * WD, N], ZDT, isOutput=False)
    wt_ext = nc.declare_dram_parameter("wt", [BPC * WD, N], F32, isOutput=True)
    # bounce buffer for the partition<->free shuffle of N: [b8, u, g, v]
    lt_dram = nc.dram_tensor("lt_scratch", [8, WD, WD, WD], F16)

    MULT = mybir.AluOpType.mult
    BYPASS = mybir.AluOpType.bypass

    with tile.TileContext(nc) as tc:
        with tc.tile_pool(name="pro", bufs=1) as pro, \
             tc.tile_pool(name="chain", bufs=2) as ch, \
             tc.tile_pool(name="zin", bufs=3) as zpool, \
             tc.tile_pool(name="wout", bufs=3) as wpool, \
             tc.tile_pool(name="cpsp", bufs=3, space="PSUM") as cpsp, \
             tc.tile_pool(name="mpsp", bufs=4, space="PSUM") as mpsp:

            # ---------- prologue: N = D^{-1} L (block-diag packed) ----------
            an = pro.tile([BPC, WD * WD], F32)
            nc.gpsimd.dma_start(out=an, in_=an_ext[:])
            cs = pro.tile([BPC, WD], F32)
            nc.gpsimd.dma_start(out=cs, in_=cs_ext[:])

            d = pro.tile([BPC, WD], F32)
            nc.scalar.sqrt(d, cs)
            r = pro.tile([BPC, WD], F32)
            nc.vector.reciprocal(r, d)

            # Ltil[b, 16u+v] = r_u * A[b,u,v], strictly-lower masked
            ltil = pro.tile([BPC, WD * WD], F16)
            for u in range(WD):
                eng = nc.vector if u % 2 == 0 else nc.gpsimd
                eng.tensor_scalar_mul(
                    ltil[:, WD * u:WD * (u + 1)],
                    an[:, WD * u:WD * (u + 1)],
                    r[:, u:u + 1])
            nc.gpsimd.affine_select(
                out=ltil, in_=ltil,
                compare_op=mybir.AluOpType.is_ge, fill=0.0,
                base=-1, pattern=[[1, WD], [-1, WD]], channel_multiplier=0)

            ident = pro.tile([BPC, BPC], F32)
            make_identity(nc, ident)
            identh = pro.tile([BPC, BPC], F16)
            make_identity(nc, identh)

            # block-diag N tile for all 16 groups: [128, 16*128]
            nbd = pro.tile([BPC, 16 * BPC], F16)
            nc.vector.memset(nbd, 0.0)
            for b8 in range(8):
                psl = slice(WD * b8, WD * (b8 + 1))
                nc.gpsimd.dma_start(
                    out=lt_dram[b8].rearrange("u g v -> g u v"),
                    in_=ltil[psl, :].rearrange("g (u v) -> g u v", u=WD),
                )
            for b8 in range(8):
                psl = slice(WD * b8, WD * (b8 + 1))
                ndst = nbd[psl, :].rearrange("u (g v) -> u g v", g=16)
                nc.gpsimd.dma_start(
                    out=ndst[:, :, WD * b8:WD * (b8 + 1)],
                    in_=lt_dram[b8],
                )

            lhsT_all = pro.tile([BPC, 16 * BPC], ZDT)
            s_all = pro.tile([BPC, 16], F32)

            def mm(lhsT, rhs):
                ps = cpsp.tile([BPC, BPC], F32, tag="cps")
                nc.tensor.matmul(ps, lhsT=lhsT, rhs=rhs, start=True, stop=True)
                return ps

            def mm_sb(lhsT, rhs, out=None):
                ps = mm(lhsT, rhs)
                if out is None:
                    out = ch.tile([BPC, BPC], F16, tag="mmsb")
                nc.vector.tensor_copy(out, ps)
                return out

            def isub(x, tag):
                out = ch.tile([BPC, BPC], F16, tag=tag)
                nc.gpsimd.tensor_sub(out, identh, x)
                return out

            def iadd(x, tag):
                out = ch.tile([BPC, BPC], F16, tag=tag)
                nc.gpsimd.tensor_add(out, identh, x)
                return out

            def vtr(x, tag):
                # blockwise 32x32 transpose == true transpose for
                # block-diag(16) operands (off-diag 32-blocks are zero)
                out = ch.tile([BPC, BPC], F16, tag=tag)
                nc.vector.transpose(out, x)
                return out

            for g in range(16):
                gsl = slice(BPC * g, BPC * (g + 1))
                ng = nbd[:, gsl]

                # ---- chain: P = (I-N)(I+N^2)(I+N^4)(I+N^8) = inv(I+N)
                # (alternating geometric series; exact since N^16 = 0)
                tg = vtr(ng, "tg")            # N^T
                s1t = isub(tg, "s1t")         # I - N^T = (I-N)^T
                n2 = mm_sb(tg, ng)            # N^2
                t2 = vtr(n2, "t2")
                a2 = iadd(n2, "a2")           # I + N^2
                n4 = mm_sb(t2, n2)            # N^4
                t4 = vtr(n4, "t4")
                a4 = iadd(n4, "a4")
                n8 = mm_sb(t4, n4)            # N^8
                a8 = iadd(n8, "a8")
                q2 = mm_sb(s1t, a2)           # (I-N)(I+N^2)
                q2t = vtr(q2, "q2t")
                q4 = mm_sb(q2t, a4)           # (I-N)(I+N^2)(I+N^4)
                q4t = vtr(q4, "q4t")
                p = mm_sb(q4t, a8, out=lhsT_all[:, gsl])   # P
                gram = mm(p, p)               # G = P^T P (PSUM)
                scr = ch.tile([BPC, BPC], F32, tag="scr")
                d2 = ch.tile([BPC, 1], F32, tag="d2")
                nc.vector.scalar_tensor_tensor(
                    out=scr, in0=gram, scalar=1.0, in1=ident,
                    op0=BYPASS, op1=MULT, accum_out=d2)
                ssq = ch.tile([BPC, 1], F32, tag="ssq")
                nc.scalar.sqrt(ssq, d2)
                nc.vector.reciprocal(s_all[:, g:g + 1], ssq)

                if g == 0 and os.environ.get("PRIORW_DEBUG"):
                    for nm, tl in [("dbg_tg", tg), ("dbg_n2", n2),
                                   ("dbg_t2", t2), ("dbg_n8", n8),
                                   ("dbg_q2", q2), ("dbg_a8", a8)]:
                        dt_ = nc.dram_tensor(nm, [BPC, BPC], F16)
                        nc.sync.dma_start(out=dt_[:], in_=tl)

                # ---- main stream for this group
                zin = zpool.tile([BPC, N], ZDT)
                nc.sync.dma_start(out=zin, in_=zt_ext[gsl, :])
                wout = wpool.tile([BPC, N], F32)
                lhs_g = lhsT_all[:, gsl]
                for t in range(N // NT):
                    csl = slice(NT * t, NT * (t + 1))
                    ps = mpsp.tile([BPC, NT], F32, tag="mps")
                    nc.tensor.matmul(ps, lhsT=lhs_g, rhs=zin[:, csl],
                                     start=True, stop=True)
                    nc.vector.tensor_scalar_mul(wout[:, csl], ps, s_all[:, g:g + 1])
                    if t == (N // NT) // 2 - 1:
                        nc.scalar.dma_start(
                            out=wt_ext[gsl, 0:N // 2], in_=wout[:, 0:N // 2])
                nc.scalar.dma_start(
                    out=wt_ext[gsl, N // 2:N], in_=wout[:, N // 2:N])

            if os.environ.get("PRIORW_DEBUG"):
                dbg_nbd = nc.dram_tensor("dbg_nbd", [BPC, 16 * BPC], F16)
                dbg_lhs = nc.dram_tensor("dbg_lhs", [BPC, 16 * BPC], ZDT)
                dbg_s = nc.dram_tensor("dbg_s", [BPC, 16], F32)
                nc.sync.dma_start(out=dbg_nbd[:], in_=nbd)
                nc.sync.dma_start(out=dbg_lhs[:], in_=lhsT_all)
                nc.sync.dma_start(out=dbg_s[:], in_=s_all)

    nc.compile()
    return nc


def _get_program():
    if "nc" not in _PROGRAM_CACHE:
        _PROGRAM_CACHE["nc"] = _build_program()
    return _PROGRAM_CACHE["nc"]


def kernel(A_noise, chisq, z):
    from concourse.bass_utils import run_bass_kernel_spmd

    A_noise = np.ascontiguousarray(A_noise, dtype=np.float32)
    chisq = np.ascontiguousarray(chisq, dtype=np.float32)
    z = np.ascontiguousarray(z, dtype=np.float32)

    trace = bool(os.environ.get("PRIORW_TRACE"))
    if trace:
        trace = _setup_trace_hooks()

    nc = _get_program()

    in_maps = []
    for c in range(NCORES):
        sl = slice(c * BPC, (c + 1) * BPC)
        # [128,4096,16] -> (b8, g, n, j) -> (g, b8, j, n) -> [2048, 4096]
        zt = np.ascontiguousarray(
            z[sl].reshape(8, 16, N, WD).transpose(1, 0, 3, 2)
        ).reshape(BPC * WD, N)
        in_maps.append({
            "a_noise": A_noise[sl].reshape(BPC, WD * WD),
            "chisq": chisq[sl],
            "zt": zt,
        })

    res = run_bass_kernel_spmd(nc, in_maps, list(range(NCORES)), trace=trace)
    if trace and res.exec_time_ns is not None:
        print(f"HW exec time: {res.exec_time_ns} ns")

    w = np.empty((NBATCH, N, WD), dtype=np.float32)
    for c in range(NCORES):
        wt = np.asarray(res.results[c]["wt"])
        w[c * BPC:(c + 1) * BPC] = (
            wt.reshape(16, 8, WD, N).transpose(1, 0, 3, 2).reshape(BPC, N, WD)
        )
    return w


# revision 20
# speedup vs baseline: 1.3322x; 1.0401x over previous
"""Trainium2 Bass kernel for nn_PriorW (Wishart-prior sampling).

Math per batch b (wdim=16):
  A  = tril(A_noise,-1) + diag(sqrt(chisq))        (lower-triangular)
  B  = inv(A),  scale_i = 1/||B[:,i]||_2
  M^T[j,i] = B[j,i]*scale_i
  w[b,n,:] = z[b,n,:] @ M^T                         (n = 4096 draws)

Device algorithm:
  A = D(I + N) with N = D^{-1} L strictly lower nilpotent (N^16 = 0), so
  (I+N)^{-1} = (I-N)(I-N^2)(I-N^4)(I-N^8) EXACTLY, i.e. P = (I+N)^{-1}
  comes from a short chain of 16x16 matrix products. Those run on the PE
  with 8 batches packed per 128x128 block-diagonal tile. Since
  B[:,i] = P[:,i]*r_i, the row normalization reduces to
  Mt[j,i] = P[j,i]/||P[:,i]|| — the final column norm is read off the
  diagonal of G = P^T P (one more PE matmul) and applied as a per-partition
  scale fused into the PSUM->SBUF copy of the main z-stream matmuls.

Sharding: pure data parallel over nbatch (1024 / 8 cores = 128 per core).
Group g in [0,16), slot b8 in [0,8): batch b = 16*b8 + g.
  zt row (g*128 + 16*b8 + j) = z[b, :, j]   (n contiguous -> fast DMA)
  wt row (g*128 + 16*b8 + i) = w[b, :, i]
"""
import os
import sys
import types

import numpy as np

for _p in ("/opt/trn_rl_repo", "/root/.axon_site/_ro/trn_rl_repo"):
    if _p not in sys.path:
        sys.path.append(_p)

import concourse.bass as bass  # noqa: E402
import concourse.tile as tile  # noqa: E402
from concourse import bacc, mybir  # noqa: E402
from concourse import bass_utils  # noqa: E402
from concourse.masks import make_identity  # noqa: E402

WD = 16          # wishart dim
BPC = 128        # batches per core
N = 4096         # draws per batch
NT = 512         # matmul moving-dim tile
NCORES = 8
NBATCH = BPC * NCORES
F32 = mybir.dt.float32
F32R = mybir.dt.float32r
F16 = mybir.dt.float16
BF16 = mybir.dt.bfloat16

# dtype for the z stream + stationary: bf16 halves the dominant read
# traffic (z is 256MB of the 512MB total); tolerance budget is ~2e-2.
_ZMODE = os.environ.get("PRIORW_ZDT", "bf16")
ZDT = {"bf16": BF16, "f32r": F32R, "f32": F32}[_ZMODE]

_PROGRAM_CACHE = {}


def _setup_trace_hooks():
    """Register the axon NTFF profile hook (missing antenv.axon_hooks shim)."""
    try:
        import antenv
        if "antenv.axon_hooks" not in sys.modules:
            hooks = types.ModuleType("antenv.axon_hooks")
            _h = [None]
            hooks.set_axon_ntff_profile_hook = lambda h: _h.__setitem__(0, h)
            hooks.get_axon_ntff_profile_hook = lambda: _h[0]
            sys.modules["antenv.axon_hooks"] = hooks
            antenv.axon_hooks = hooks
        from antenv.axon_hooks import set_axon_ntff_profile_hook
        from trn_agent_boot.trn_boot import _ntff_profile_via_ctypes
        hook = _ntff_profile_via_ctypes("/opt/axon/libaxon_pjrt.so")
        if hook is not None:
            set_axon_ntff_profile_hook(hook)
        bass_utils.upload_artifacts = lambda tmpdir: tmpdir  # no egress
        return True
    except Exception:
        return False


def _build_program():
    nc = bacc.Bacc("TRN2", target_bir_lowering=False)
    an_ext = nc.declare_dram_parameter("a_noise", [BPC, WD * WD], F32, isOutput=False)
    cs_ext = nc.declare_dram_parameter("chisq", [BPC, WD], F32, isOutput=False)
    zt_ext = nc.declare_dram_parameter("zt", [BPC * WD, N], ZDT, isOutput=False)
    wt_ext = nc.declare_dram_parameter("wt", [BPC * WD, N], F32, isOutput=True)
    # bounce buffer for the partition<->free shuffle of N: [b8, u, g, v]
    lt_dram = nc.dram_tensor("lt_scratch", [8, WD, WD, WD], F16)

    MULT = mybir.AluOpType.mult
    BYPASS = mybir.AluOpType.bypass

    with tile.TileContext(nc) as tc:
        with tc.tile_pool(name="pro", bufs=1) as pro, \
             tc.tile_pool(name="chain", bufs=2) as ch, \
             tc.tile_pool(name="zin", bufs=5) as zpool, \
             tc.tile_pool(name="wout", bufs=3) as wpool, \
             tc.tile_pool(name="cpsp", bufs=3, space="PSUM") as cpsp, \
             tc.tile_pool(name="mpsp", bufs=4, space="PSUM") as mpsp:

            # ---------- prologue: N = D^{-1} L (block-diag packed) ----------
            an = pro.tile([BPC, WD * WD], F32)
            nc.sync.dma_start(out=an, in_=an_ext[:])
            cs = pro.tile([BPC, WD], F32)
            nc.sync.dma_start(out=cs, in_=cs_ext[:])

            d = pro.tile([BPC, WD], F32)
            nc.scalar.sqrt(d, cs)
            r = pro.tile([BPC, WD], F32)
            nc.vector.reciprocal(r, d)

            # Ltil[b, 16u+v] = r_u * A[b,u,v], strictly-lower masked
            ltil = pro.tile([BPC, WD * WD], F16)
            for u in range(WD):
                eng = nc.vector
                eng.tensor_scalar_mul(
                    ltil[:, WD * u:WD * (u + 1)],
                    an[:, WD * u:WD * (u + 1)],
                    r[:, u:u + 1])
            nc.gpsimd.affine_select(
                out=ltil, in_=ltil,
                compare_op=mybir.AluOpType.is_ge, fill=0.0,
                base=-1, pattern=[[1, WD], [-1, WD]], channel_multiplier=0)

            ident = pro.tile([BPC, BPC], F32)
            make_identity(nc, ident)
            identh = pro.tile([BPC, BPC], F16)
            make_identity(nc, identh)

            # block-diag N tile for all 16 groups: [128, 16*128]
            nbd = pro.tile([BPC, 16 * BPC], F16)
            nc.vector.memset(nbd, 0.0)
            for b8 in range(8):
                psl = slice(WD * b8, WD * (b8 + 1))
                nc.sync.dma_start(
                    out=lt_dram[b8].rearrange("u g v -> g u v"),
                    in_=ltil[psl, :].rearrange("g (u v) -> g u v", u=WD),
                )
            for b8 in range(8):
                psl = slice(WD * b8, WD * (b8 + 1))
                ndst = nbd[psl, :].rearrange("u (g v) -> u g v", g=16)
                nc.sync.dma_start(
                    out=ndst[:, :, WD * b8:WD * (b8 + 1)],
                    in_=lt_dram[b8],
                )

            lhsT_all = pro.tile([BPC, 16 * BPC], ZDT)
            s_all = pro.tile([BPC, 16], F32)

            def mm(lhsT, rhs):
                ps = cpsp.tile([BPC, BPC], F32, tag="cps")
                nc.tensor.matmul(ps, lhsT=lhsT, rhs=rhs, start=True, stop=True)
                return ps

            def mm_sb(lhsT, rhs, out=None):
                ps = mm(lhsT, rhs)
                if out is None:
                    out = ch.tile([BPC, BPC], F16, tag="mmsb")
                nc.vector.tensor_copy(out, ps)
                return out

            def isub(x, tag):
                out = ch.tile([BPC, BPC], F16, tag=tag)
                nc.gpsimd.tensor_sub(out, identh, x)
                return out

            def iadd(x, tag):
                out = ch.tile([BPC, BPC], F16, tag=tag)
                nc.gpsimd.tensor_add(out, identh, x)
                return out

            def vtr(x, tag):
                # blockwise 32x32 transpose == true transpose for
                # block-diag(16) operands (off-diag 32-blocks are zero)
                out = ch.tile([BPC, BPC], F16, tag=tag)
                nc.vector.transpose(out, x)
                return out

            for g in range(16):
                gsl = slice(BPC * g, BPC * (g + 1))
                ng = nbd[:, gsl]

                # ---- chain: P = (I-N)(I+N^2)(I+N^4)(I+N^8) = inv(I+N)
                # (alternating geometric series; exact since N^16 = 0)
                tg = vtr(ng, "tg")            # N^T
                s1t = isub(tg, "s1t")         # I - N^T = (I-N)^T
                n2 = mm_sb(tg, ng)            # N^2
                t2 = vtr(n2, "t2")
                a2 = iadd(n2, "a2")           # I + N^2
                n4 = mm_sb(t2, n2)            # N^4
                t4 = vtr(n4, "t4")
                a4 = iadd(n4, "a4")
                n8 = mm_sb(t4, n4)            # N^8
                a8 = iadd(n8, "a8")
                q2 = mm_sb(s1t, a2)           # (I-N)(I+N^2)
                q2t = vtr(q2, "q2t")
                q4 = mm_sb(q2t, a4)           # (I-N)(I+N^2)(I+N^4)
                q4t = vtr(q4, "q4t")
                p = mm_sb(q4t, a8, out=lhsT_all[:, gsl])   # P
                gram = mm(p, p)               # G = P^T P (PSUM)
                scr = ch.tile([BPC, BPC], F32, tag="scr")
                d2 = ch.tile([BPC, 1], F32, tag="d2")
                nc.vector.scalar_tensor_tensor(
                    out=scr, in0=gram, scalar=1.0, in1=ident,
                    op0=BYPASS, op1=MULT, accum_out=d2)
                ssq = ch.tile([BPC, 1], F32, tag="ssq")
                nc.scalar.sqrt(ssq, d2)
                nc.vector.reciprocal(s_all[:, g:g + 1], ssq)

                if g == 0 and os.environ.get("PRIORW_DEBUG"):
                    for nm, tl in [("dbg_tg", tg), ("dbg_n2", n2),
                                   ("dbg_t2", t2), ("dbg_n8", n8),
                                   ("dbg_q2", q2), ("dbg_a8", a8)]:
                        dt_ = nc.dram_tensor(nm, [BPC, BPC], F16)
                        nc.sync.dma_start(out=dt_[:], in_=tl)

                # ---- main stream for this group
                zin = zpool.tile([BPC, N], ZDT)
                nc.sync.dma_start(out=zin, in_=zt_ext[gsl, :])
                wout = wpool.tile([BPC, N], F32)
                lhs_g = lhsT_all[:, gsl]
                for t in range(N // NT):
                    csl = slice(NT * t, NT * (t + 1))
                    ps = mpsp.tile([BPC, NT], F32, tag="mps")
                    nc.tensor.matmul(ps, lhsT=lhs_g, rhs=zin[:, csl],
                                     start=True, stop=True)
                    nc.vector.tensor_scalar_mul(wout[:, csl], ps, s_all[:, g:g + 1])
                    if t == (N // NT) // 2 - 1:
                        nc.scalar.dma_start(
                            out=wt_ext[gsl, 0:N // 2], in_=wout[:, 0:N // 2])
                nc.scalar.dma_start(
                    out=wt_ext[gsl, N // 2:N], in_=wout[:, N // 2:N])

            if os.environ.get("PRIORW_DEBUG"):
                dbg_nbd = nc.dram_tensor("dbg_nbd", [BPC, 16 * BPC], F16)
                dbg_lhs = nc.dram_tensor("dbg_lhs", [BPC, 16 * BPC], ZDT)
                dbg_s = nc.dram_tensor("dbg_s", [BPC, 16], F32)
                nc.sync.dma_start(out=dbg_nbd[:], in_=nbd)
                nc.sync.dma_start(out=dbg_lhs[:], in_=lhsT_all)
                nc.sync.dma_start(out=dbg_s[:], in_=s_all)

    nc.compile()
    return nc


def _get_program():
    if "nc" not in _PROGRAM_CACHE:
        _PROGRAM_CACHE["nc"] = _build_program()
    return _PROGRAM_CACHE["nc"]


def kernel(A_noise, chisq, z):
    from concourse.bass_utils import run_bass_kernel_spmd

    A_noise = np.ascontiguousarray(A_noise, dtype=np.float32)
    chisq = np.ascontiguousarray(chisq, dtype=np.float32)
    z = np.ascontiguousarray(z, dtype=np.float32)

    trace = bool(os.environ.get("PRIORW_TRACE"))
    if trace:
        trace = _setup_trace_hooks()

    nc = _get_program()

    import ml_dtypes
    z_np_dt = {BF16: ml_dtypes.bfloat16, F32R: np.float32, F32: np.float32}[ZDT]

    in_maps = []
    for c in range(NCORES):
        sl = slice(c * BPC, (c + 1) * BPC)
        # [128,4096,16] -> (b8, g, n, j) -> (g, b8, j, n) -> [2048, 4096]
        zt = np.ascontiguousarray(
            z[sl].reshape(8, 16, N, WD).transpose(1, 0, 3, 2)
        ).reshape(BPC * WD, N).astype(z_np_dt)
        in_maps.append({
            "a_noise": A_noise[sl].reshape(BPC, WD * WD),
            "chisq": chisq[sl],
            "zt": zt,
        })

    res = run_bass_kernel_spmd(nc, in_maps, list(range(NCORES)), trace=trace)
    if trace and res.exec_time_ns is not None:
        print(f"HW exec time: {res.exec_time_ns} ns")

    w = np.empty((NBATCH, N, WD), dtype=np.float32)
    for c in range(NCORES):
        wt = np.asarray(res.results[c]["wt"])
        w[c * BPC:(c + 1) * BPC] = (
            wt.reshape(16, 8, WD, N).transpose(1, 0, 3, 2).reshape(BPC, N, WD)
        )
    return w
